# revision 10
# baseline (speedup 1.0000x reference)
"""Trainium2 Bass kernel for nn_DynamicReceptiveEncoder (v2).

Structure (per core, 4 of 32 batch elements):
  PE  : conv3+conv7 as one K=50 matmul (bias via ones row, f32) feeding the
        A-side scan via PSUM; diff-conv (K=49, fp32r, x-diff done on host,
        1.25 threshold-normalization folded into weights) feeding B-side.
  GPS : exact 512-step LIF scan for the two raw-conv neurons (tau 20/50,
        long subthreshold memory - cannot be time-chunked), one
        scalar_tensor_tensor pair per step, reading conv PSUM directly.
  DVE : time-chunked LIF scan for the two |diff| neurons (tau 2/0.91,
        state forgets within ~10 steps): 8 chunks x (64+16) steps
        processed as 800 columns per instruction.
  ACT : |.| eviction of diff-conv PSUM; Sign(v-1) spike masks for both
        sides (bf16) which are DMA'd to DRAM.
  Host: im2col staging matrices (so device DMA is wide contiguous block
        copy), final spike summation across the four neuron masks.
"""

import sys

sys.path.insert(0, "/opt/trn_rl_repo")

import numpy as np

import concourse.bass as bass
import concourse.mybir as mybir
from concourse.tile import TileContext
from concourse import bass_utils

AL = mybir.AluOpType
AF = mybir.ActivationFunctionType
F32 = mybir.dt.float32
F32R = mybir.dt.float32r
BF16 = mybir.dt.bfloat16

# ---------------------------------------------------------------------------
# Patches for this walrus build (max ONE sync wait per instruction) and for
# the missing NTFF profile hook module.
# ---------------------------------------------------------------------------
import concourse.tile as _tile
from concourse.vector_clock import ScopedClock as _ScopedClock

_wsplit_counter = [0]


def _patched_drain_and_barrier(self, tick_clock, wait_clock):
    nc = self.nc
    drain_inst = nc.sync.drain()
    wait_clock.add_sem_waits(
        drain_inst.ins, _ScopedClock({None: tick_clock.global_clock})
    )
    si = drain_inst.ins.sync_info
    waits = list(si.on_wait) if si is not None else []
    if len(waits) > 1:
        updates = list(si.on_update) if si is not None else []
        drain_inst.ins.sync_info = mybir.SyncInfo(on_wait=[], on_update=updates)
        for w in waits:
            nop_inst = nc.sync.nop(nofuse=True)
            nop_inst.ins.sync_info = mybir.SyncInfo(on_wait=[w], on_update=[])

    nc.all_engine_barrier()
    assert self.sems is not None
    popped = nc._tile_sem_poison_stack.pop()
    assert popped is self._sem_poison
    nc.clear_and_free_semaphores(list(self.sems.allocated().values()))
    nc.all_engine_barrier()


_tile.TileContext._drain_and_barrier = _patched_drain_and_barrier


def _split_multi_waits(nc, max_waits=1):
    for f in nc.m.functions:
        for bb in f.blocks:
            insts = bb.instructions
            i = 0
            while i < len(insts):
                inst = insts[i]
                si = inst.sync_info
                if si is not None and len(si.on_wait) > max_waits:
                    waits = list(si.on_wait)
                    extra, keep = waits[:-max_waits], waits[-max_waits:]
                    inst.sync_info = mybir.SyncInfo(
                        on_wait=keep, on_update=list(si.on_update)
                    )
                    for w in extra:
                        _wsplit_counter[0] += 1
                        nop = mybir.InstNoOp(
                            name=f"wsplit_{_wsplit_counter[0]}", ins=[], outs=[]
                        )
                        nop.engine = inst.engine
                        nop.sync_info = mybir.SyncInfo(on_wait=[w], on_update=[])
                        insts.insert(i, nop)
                        i += 1
                i += 1


def _install_ntff_hook():
    import contextlib, ctypes, types

    try:
        lib = ctypes.CDLL("/opt/axon/libaxon_pjrt.so")
    except OSError:
        return
    if not hasattr(lib, "axon_start_nrt_profile"):
        return
    lib.axon_start_nrt_profile.argtypes = [
        ctypes.POINTER(ctypes.c_int64),
        ctypes.c_size_t,
    ]
    lib.axon_start_nrt_profile.restype = ctypes.c_int64
    lib.axon_stop_nrt_profile.argtypes = [ctypes.c_char_p]
    lib.axon_stop_nrt_profile.restype = ctypes.c_int64

    @contextlib.contextmanager
    def _hook(output_dir, device_ids):
        import jax

        jax.devices()
        if device_ids:
            ids = (ctypes.c_int64 * len(device_ids))(*device_ids)
            rc = lib.axon_start_nrt_profile(ids, len(device_ids))
        else:
            rc = lib.axon_start_nrt_profile(None, 0)
        if rc != 0:
            raise RuntimeError(f"axon_start_nrt_profile rc={rc}")
        try:
            yield
        finally:
            lib.axon_stop_nrt_profile(str(output_dir).encode())

    mod = types.ModuleType("antenv.axon_hooks")
    holder = [_hook]
    mod.set_axon_ntff_profile_hook = lambda h: holder.__setitem__(0, h)
    mod.get_axon_ntff_profile_hook = lambda: holder[0]
    sys.modules["antenv.axon_hooks"] = mod
    try:
        import antenv

        antenv.axon_hooks = mod
    except ImportError:
        pass


_install_ntff_hook()

# ---------------------------------------------------------------------------
# Problem constants
# ---------------------------------------------------------------------------
B, W, F, C = 32, 512, 25, 64
NCORES = 8
BL = B // NCORES            # 4 batch elements per core
NBF = BL * F                # 100 (f, b) columns
KA = 50                     # 49 taps + bias/ones row
KD = 49                     # diff-conv taps only (bias cancels)

PB = 8                      # B-side time chunks
CH = W // PB                # 64 steps per chunk
LB = 24                     # B-side warmup steps
SB = CH + LB                # 80 sequential B steps
NB = PB * NBF               # 800 B-side columns per step

TWA = 16                    # A-side staging window (steps)
TWD = 8                     # B-side staging window (B-steps, SB=88 -> 11 windows)

TAU = (20.0, 50.0, 2.0, 0.91)
ALPHA = tuple(np.float32(1.0 - 1.0 / t) for t in TAU)


def _build_nc():
    nc = bass.Bass()
    patA = nc.dram_tensor("patA", [KA, W * NBF], F32, kind="ExternalInput")
    patD = nc.dram_tensor("patD", [KD, SB * NB], F32R, kind="ExternalInput")
    wAt = nc.dram_tensor("wA", [KA, 128], F32, kind="ExternalInput")
    wDt = nc.dram_tensor("wD", [KD, 128], F32R, kind="ExternalInput")
    alAt = nc.dram_tensor("alphaA", [128, 1], F32, kind="ExternalInput")
    alBt = nc.dram_tensor("alphaB", [128, 1], F32, kind="ExternalInput")
    vAo = nc.dram_tensor("vA", [W // 8, 128, 8 * NBF], F32, kind="ExternalOutput")
    vBo = nc.dram_tensor("vB", [CH, 128, NB], F32, kind="ExternalOutput")

    with TileContext(nc) as tc:
        with (
            tc.tile_pool(name="consts", bufs=1) as cpool,
            tc.tile_pool(name="vA", bufs=3) as vApool,
            tc.tile_pool(name="vB", bufs=3) as vBpool,
            tc.tile_pool(name="xA", bufs=3) as xApool,
            tc.tile_pool(name="xB", bufs=3) as xBpool,
            tc.tile_pool(name="psA", bufs=4, space="PSUM") as psApool,
            tc.tile_pool(name="psB", bufs=2, space="PSUM") as psBpool,
        ):
            wA_sb = cpool.tile([KA, 128], F32, name="wA_sb")
            nc.sync.dma_start(wA_sb[:], wAt[:])
            wD_sb = cpool.tile([KD, 128], F32R, name="wD_sb")
            nc.sync.dma_start(wD_sb[:], wDt[:])
            alA = cpool.tile([128, 1], F32, name="alA")
            nc.sync.dma_start(alA[:], alAt[:])
            alB = cpool.tile([128, 1], F32, name="alB")
            nc.sync.dma_start(alB[:], alBt[:])
            wAst = cpool.tile([128, NBF], F32, name="wAst")
            nc.vector.memset(wAst[:], 0.0)
            wBst = cpool.tile([128, NB], F32, name="wBst")
            nc.vector.memset(wBst[:], 0.0)

            # staging buffers (persistent, manual rotation; Tile serializes
            # WAR on reuse)
            rhsA = [cpool.tile([KA, TWA * NBF], F32, name=f"rhsA{i}")
                    for i in range(4)]
            rhsD = [cpool.tile([KD, TWD * NB], F32R, name=f"rhsD{i}")
                    for i in range(3)]

            def stage_A(w):
                eng = nc.sync if w % 2 == 0 else nc.scalar
                eng.dma_start(
                    rhsA[w % 4][:],
                    patA[:, w * TWA * NBF:(w + 1) * TWA * NBF],
                )

            def stage_D(w):
                nc.gpsimd.dma_start(
                    rhsD[w % 3][:],
                    patD[:, w * TWD * NB:(w + 1) * TWD * NB],
                )

            psA_tiles = {}
            xA_tiles = {}

            def conv_A(g):
                # conv for steps 4g..4g+3 (N=400), full fp32 for exactness
                ps = psApool.tile([128, 512], F32, name="psA")
                psA_tiles[g] = ps
                w = g // 4
                sl = (g % 4) * 4 * NBF
                nc.tensor.matmul(
                    ps[:, 0:4 * NBF],
                    wA_sb[:],
                    rhsA[w % 4][:, sl:sl + 4 * NBF],
                    start=True,
                    stop=True,
                )

            def evict_A(g):
                # PSUM -> SBUF for steps 4g..4g+3, one ACT instruction
                ps = psA_tiles.pop(g)
                xA = xApool.tile([128, 4 * NBF], F32, name="xA")
                xA_tiles[g] = xA
                nc.scalar.activation(xA[:], ps[:, 0:4 * NBF], AF.Identity,
                                     bias=0.0, scale=1.0)

            psB_tiles = {}

            def conv_B(k):
                # diff-conv for B-step k (N=800 as 2x400 in one 2-bank tile)
                w = k // TWD
                sl = (k % TWD) * NB
                ps = psBpool.tile([128, 1024], F32, name="psB")
                psB_tiles[k] = ps
                for h in range(2):
                    nc.tensor.matmul(
                        ps[:, h * 512:h * 512 + 400],
                        wD_sb[:],
                        rhsD[w % 3][:, sl + h * 400: sl + (h + 1) * 400],
                        start=True,
                        stop=True,
                    )

            vA_tiles = {}
            vB_tiles = {}

            def scan_A(t):
                blk = t // 8
                if t % 8 == 0:
                    vA_tiles[blk] = vApool.tile([128, 8 * NBF], F32, name="vA")
                vA = vA_tiles[blk]
                sl = (t % 8) * NBF
                xA = xA_tiles[t // 4]
                nc.vector.scalar_tensor_tensor(
                    out=vA[:, sl:sl + NBF],
                    in0=wAst[:],
                    scalar=alA[:],
                    in1=xA[:, (t % 4) * NBF:(t % 4 + 1) * NBF],
                    op0=AL.mult,
                    op1=AL.add,
                )
                nc.vector.scalar_tensor_tensor(
                    out=wAst[:],
                    in0=vA[:, sl:sl + NBF],
                    scalar=1.0,
                    in1=vA[:, sl:sl + NBF],
                    op0=AL.is_lt,
                    op1=AL.mult,
                )
                if t % 4 == 3:
                    xA_tiles.pop(t // 4, None)
                if t % 8 == 7:
                    eng = nc.sync if blk % 2 == 0 else nc.scalar
                    eng.dma_start(vAo[blk], vA[:])

            def scan_B(k):
                ps = psB_tiles.pop(k)
                xB = xBpool.tile([128, NB], F32, name="xB")
                src_ap = bass.AP(
                    tensor=ps.tensor,
                    offset=ps.offset,
                    ap=[list(ps.ap[0]), [512, 2], [1, 400]],
                )
                dst_ap = bass.AP(
                    tensor=xB.tensor,
                    offset=xB.offset,
                    ap=[list(xB.ap[0]), [400, 2], [1, 400]],
                )
                nc.scalar.activation(dst_ap, src_ap, AF.Abs,
                                     bias=0.0, scale=1.0)
                if k == LB:
                    # t=0 column block (c=0): temporal delta is defined as 0
                    nc.vector.memset(xB[:, 0:NBF], 0.0)
                vB = vBpool.tile([128, NB], F32, name="vB")
                vB_tiles[k] = vB
                nc.vector.scalar_tensor_tensor(
                    out=vB[:],
                    in0=wBst[:],
                    scalar=alB[:],
                    in1=xB[:],
                    op0=AL.mult,
                    op1=AL.add,
                )
                nc.vector.scalar_tensor_tensor(
                    out=wBst[:],
                    in0=vB[:],
                    scalar=1.0,
                    in1=vB[:],
                    op0=AL.is_lt,
                    op1=AL.mult,
                )
                if k >= LB:
                    eng = nc.scalar if k % 2 == 0 else nc.sync
                    eng.dma_start(vBo[k - LB], vB[:])

            # prologue: stage first windows, first convs + evicts
            stage_A(0)
            stage_D(0)
            stage_A(1)
            stage_D(1)
            stage_A(2)
            for g in range(3):
                conv_A(g)
            evict_A(0)
            evict_A(1)
            conv_B(0)

            kB = 0
            for t in range(W):
                if t % TWA == 0 and t // TWA + 3 < W // TWA:
                    stage_A(t // TWA + 3)
                if t % 4 == 0:
                    g = t // 4
                    if g + 3 < 128:
                        conv_A(g + 3)
                    if g + 2 < 128:
                        evict_A(g + 2)
                scan_A(t)
                k_target = ((t + 1) * SB) // W
                while kB < min(k_target, SB):
                    k = kB
                    if k % TWD == 0 and k // TWD + 2 < SB // TWD:
                        stage_D(k // TWD + 2)
                    if k + 1 < SB:
                        conv_B(k + 1)
                    scan_B(k)
                    kB += 1

    _split_multi_waits(nc)
    return nc


_NC_CACHE = [None]
LAST_RESULT = [None]


def _get_nc():
    if _NC_CACHE[0] is None:
        _NC_CACHE[0] = _build_nc()
    return _NC_CACHE[0]


def _host_prep(inputs, w3, b3, w7, b7):
    f32 = np.float32
    swv = np.lib.stride_tricks.sliding_window_view

    # weight matrices [K, 128]; cols 0:64 conv3 channels, 64:128 conv7
    wA = np.zeros((KA, 128), dtype=f32)
    w3r = w3.reshape(C, 3, 3)
    w7r = w7.reshape(C, 7, 7)
    for i in range(3):
        for j in range(3):
            wA[(i + 2) * 7 + (j + 2), 0:C] = w3r[:, i, j]
    for i in range(7):
        for j in range(7):
            wA[i * 7 + j, C:128] = w7r[:, i, j]
    wA[49, 0:C] = b3
    wA[49, C:128] = b7
    wD = (wA[:49] * f32(1.25)).astype(f32)   # threshold-normalized diff conv

    alphaA = np.full((128, 1), ALPHA[0], dtype=f32)
    alphaA[C:, 0] = ALPHA[1]
    alphaB = np.full((128, 1), ALPHA[2], dtype=f32)
    alphaB[C:, 0] = ALPHA[3]

    per_core = []
    for ci in range(NCORES):
        xc = inputs[ci * BL:(ci + 1) * BL]          # [4, 512, 25] (b, t, f)

        # A-side im2col: patA[(i*7+j), t*100 + f*4 + b] = xpad[b, t+j, f+i]
        xp = np.zeros((BL, W + 6, F + 6), dtype=f32)
        xp[:, 3:3 + W, 3:3 + F] = xc
        v = swv(xp, (7, 7), axis=(1, 2))            # [b, t, f, j, i]
        pA = np.ascontiguousarray(v.transpose(4, 3, 1, 2, 0))  # [i,j,t,f,b]
        patA = np.empty((KA, W * NBF), dtype=f32)
        patA[:49] = pA.reshape(49, W * NBF)
        patA[49] = 1.0                               # bias row

        # B-side: temporal diff on the padded domain.
        # xd[tau] = x[tau] - x[tau-1] for tau in [0, 512], xd[512] = -x[511]
        xdp = np.zeros((BL, LB + 3 + W + 3, F + 6), dtype=f32)
        xd = np.diff(xc, axis=1, prepend=np.zeros_like(xc[:, :1]))
        xdp[:, LB + 3:LB + 3 + W, 3:3 + F] = xd
        xdp[:, LB + 3 + W, 3:3 + F] = -xc[:, -1]
        vD = swv(xdp, (7, 7), axis=(1, 2))          # [b, w0, f, j, i]
        pD = np.ascontiguousarray(vD.transpose(4, 3, 1, 2, 0))  # [i,j,w0,f,b]
        pD = pD.reshape(49, LB + W - CH + CH, F, BL) if False else pD
        # columns: (k, c, f, b) with w0 = c*64 + k
        idx = (np.arange(PB)[None, :] * CH + np.arange(SB)[:, None])  # [k, c]
        patD = np.ascontiguousarray(
            pD.reshape(49, -1, F, BL)[:, idx.reshape(-1)]
        ).reshape(KD, SB * NB)

        per_core.append({
            "patA": patA,
            "patD": patD,
            "wA": wA,
            "wD": wD,
            "alphaA": alphaA,
            "alphaB": alphaB,
        })
    return per_core


def _host_post(res):
    f32 = np.float32
    outs = []
    for ci in range(NCORES):
        r = res.results[ci]
        # spikes = (v >= 1), matching the reference heaviside exactly
        sA = (np.asarray(r["vA"]) >= 1.0).astype(f32)   # [blk, ch, tl*100+n]
        sB = (np.asarray(r["vB"]) >= 1.0).astype(f32)   # [k, ch, c*100+n]
        sA = sA.reshape(W // 8, 128, 8, F, BL)
        sA = sA.transpose(1, 0, 2, 3, 4).reshape(128, W, F, BL)
        sB = sB.reshape(CH, 128, PB, F, BL)
        sB = sB.transpose(1, 2, 0, 3, 4).reshape(128, W, F, BL)
        out = (sA[:C] + sA[C:] + sB[:C] + sB[C:])   # [C, t, f, b]
        outs.append(out.transpose(3, 0, 2, 1))      # [b, C, f, t]
    return np.ascontiguousarray(np.concatenate(outs, axis=0), dtype=f32)


def kernel(inputs, w3, b3, w7, b7):
    nc = _get_nc()
    per_core = _host_prep(
        np.asarray(inputs, dtype=np.float32),
        np.asarray(w3, dtype=np.float32),
        np.asarray(b3, dtype=np.float32),
        np.asarray(w7, dtype=np.float32),
        np.asarray(b7, dtype=np.float32),
    )
    res = bass_utils.run_bass_kernel_spmd(
        nc, per_core, core_ids=list(range(NCORES))
    )
    LAST_RESULT[0] = res
    return _host_post(res)


# revision 11
# speedup vs baseline: 1.0157x; 1.0157x over previous
"""Trainium2 Bass kernel for nn_DynamicReceptiveEncoder (v2).

Structure (per core, 4 of 32 batch elements):
  PE  : conv3+conv7 as one K=50 matmul (bias via ones row, f32) feeding the
        A-side scan via PSUM; diff-conv (K=49, fp32r, x-diff done on host,
        1.25 threshold-normalization folded into weights) feeding B-side.
  GPS : exact 512-step LIF scan for the two raw-conv neurons (tau 20/50,
        long subthreshold memory - cannot be time-chunked), one
        scalar_tensor_tensor pair per step, reading conv PSUM directly.
  DVE : time-chunked LIF scan for the two |diff| neurons (tau 2/0.91,
        state forgets within ~10 steps): 8 chunks x (64+16) steps
        processed as 800 columns per instruction.
  ACT : |.| eviction of diff-conv PSUM; Sign(v-1) spike masks for both
        sides (bf16) which are DMA'd to DRAM.
  Host: im2col staging matrices (so device DMA is wide contiguous block
        copy), final spike summation across the four neuron masks.
"""

import sys

sys.path.insert(0, "/opt/trn_rl_repo")

import numpy as np

import concourse.bass as bass
import concourse.mybir as mybir
from concourse.tile import TileContext
from concourse import bass_utils

AL = mybir.AluOpType
AF = mybir.ActivationFunctionType
F32 = mybir.dt.float32
F32R = mybir.dt.float32r
BF16 = mybir.dt.bfloat16

# ---------------------------------------------------------------------------
# Patches for this walrus build (max ONE sync wait per instruction) and for
# the missing NTFF profile hook module.
# ---------------------------------------------------------------------------
import concourse.tile as _tile
from concourse.vector_clock import ScopedClock as _ScopedClock

_wsplit_counter = [0]


def _patched_drain_and_barrier(self, tick_clock, wait_clock):
    nc = self.nc
    drain_inst = nc.sync.drain()
    wait_clock.add_sem_waits(
        drain_inst.ins, _ScopedClock({None: tick_clock.global_clock})
    )
    si = drain_inst.ins.sync_info
    waits = list(si.on_wait) if si is not None else []
    if len(waits) > 1:
        updates = list(si.on_update) if si is not None else []
        drain_inst.ins.sync_info = mybir.SyncInfo(on_wait=[], on_update=updates)
        for w in waits:
            nop_inst = nc.sync.nop(nofuse=True)
            nop_inst.ins.sync_info = mybir.SyncInfo(on_wait=[w], on_update=[])

    nc.all_engine_barrier()
    assert self.sems is not None
    popped = nc._tile_sem_poison_stack.pop()
    assert popped is self._sem_poison
    nc.clear_and_free_semaphores(list(self.sems.allocated().values()))
    nc.all_engine_barrier()


_tile.TileContext._drain_and_barrier = _patched_drain_and_barrier


def _split_multi_waits(nc, max_waits=1):
    for f in nc.m.functions:
        for bb in f.blocks:
            insts = bb.instructions
            i = 0
            while i < len(insts):
                inst = insts[i]
                si = inst.sync_info
                if si is not None and len(si.on_wait) > max_waits:
                    waits = list(si.on_wait)
                    extra, keep = waits[:-max_waits], waits[-max_waits:]
                    inst.sync_info = mybir.SyncInfo(
                        on_wait=keep, on_update=list(si.on_update)
                    )
                    for w in extra:
                        _wsplit_counter[0] += 1
                        nop = mybir.InstNoOp(
                            name=f"wsplit_{_wsplit_counter[0]}", ins=[], outs=[]
                        )
                        nop.engine = inst.engine
                        nop.sync_info = mybir.SyncInfo(on_wait=[w], on_update=[])
                        insts.insert(i, nop)
                        i += 1
                i += 1


def _install_ntff_hook():
    import contextlib, ctypes, types

    try:
        lib = ctypes.CDLL("/opt/axon/libaxon_pjrt.so")
    except OSError:
        return
    if not hasattr(lib, "axon_start_nrt_profile"):
        return
    lib.axon_start_nrt_profile.argtypes = [
        ctypes.POINTER(ctypes.c_int64),
        ctypes.c_size_t,
    ]
    lib.axon_start_nrt_profile.restype = ctypes.c_int64
    lib.axon_stop_nrt_profile.argtypes = [ctypes.c_char_p]
    lib.axon_stop_nrt_profile.restype = ctypes.c_int64

    @contextlib.contextmanager
    def _hook(output_dir, device_ids):
        import jax

        jax.devices()
        if device_ids:
            ids = (ctypes.c_int64 * len(device_ids))(*device_ids)
            rc = lib.axon_start_nrt_profile(ids, len(device_ids))
        else:
            rc = lib.axon_start_nrt_profile(None, 0)
        if rc != 0:
            raise RuntimeError(f"axon_start_nrt_profile rc={rc}")
        try:
            yield
        finally:
            lib.axon_stop_nrt_profile(str(output_dir).encode())

    mod = types.ModuleType("antenv.axon_hooks")
    holder = [_hook]
    mod.set_axon_ntff_profile_hook = lambda h: holder.__setitem__(0, h)
    mod.get_axon_ntff_profile_hook = lambda: holder[0]
    sys.modules["antenv.axon_hooks"] = mod
    try:
        import antenv

        antenv.axon_hooks = mod
    except ImportError:
        pass


_install_ntff_hook()

# ---------------------------------------------------------------------------
# Problem constants
# ---------------------------------------------------------------------------
B, W, F, C = 32, 512, 25, 64
NCORES = 8
BL = B // NCORES            # 4 batch elements per core
NBF = BL * F                # 100 (f, b) columns
KA = 50                     # 49 taps + bias/ones row
KD = 49                     # diff-conv taps only (bias cancels)

PB = 8                      # B-side time chunks
CH = W // PB                # 64 steps per chunk
LB = 16                     # B-side warmup steps
SB = CH + LB                # 80 sequential B steps
NB = PB * NBF               # 800 B-side columns per step

TWA = 16                    # A-side staging window (steps)
TWD = 8                     # B-side staging window (B-steps, SB=88 -> 11 windows)

TAU = (20.0, 50.0, 2.0, 0.91)
ALPHA = tuple(np.float32(1.0 - 1.0 / t) for t in TAU)


def _build_nc():
    nc = bass.Bass()
    patA = nc.dram_tensor("patA", [KA, W * NBF], F32, kind="ExternalInput")
    patD = nc.dram_tensor("patD", [KD, SB * NB], F32R, kind="ExternalInput")
    wAt = nc.dram_tensor("wA", [KA, 128], F32, kind="ExternalInput")
    wDt = nc.dram_tensor("wD", [KD, 128], F32R, kind="ExternalInput")
    alAt = nc.dram_tensor("alphaA", [128, 1], F32, kind="ExternalInput")
    alBt = nc.dram_tensor("alphaB", [128, 1], F32, kind="ExternalInput")
    vAo = nc.dram_tensor("vA", [W // 8, 128, 8 * NBF], F32, kind="ExternalOutput")
    vBo = nc.dram_tensor("vB", [CH, 128, NB], F32, kind="ExternalOutput")

    with TileContext(nc) as tc:
        with (
            tc.tile_pool(name="consts", bufs=1) as cpool,
            tc.tile_pool(name="vA", bufs=3) as vApool,
            tc.tile_pool(name="vB", bufs=3) as vBpool,
            tc.tile_pool(name="xA", bufs=3) as xApool,
            tc.tile_pool(name="xB", bufs=3) as xBpool,
            tc.tile_pool(name="psA", bufs=2, space="PSUM") as psApool,
            tc.tile_pool(name="psB", bufs=2, space="PSUM") as psBpool,
        ):
            wA_sb = cpool.tile([KA, 128], F32, name="wA_sb")
            nc.sync.dma_start(wA_sb[:], wAt[:])
            wD_sb = cpool.tile([KD, 128], F32R, name="wD_sb")
            nc.sync.dma_start(wD_sb[:], wDt[:])
            alA = cpool.tile([128, 1], F32, name="alA")
            nc.sync.dma_start(alA[:], alAt[:])
            alB = cpool.tile([128, 1], F32, name="alB")
            nc.sync.dma_start(alB[:], alBt[:])
            wAst = cpool.tile([128, NBF], F32, name="wAst")
            nc.vector.memset(wAst[:], 0.0)
            wBst = cpool.tile([128, NB], F32, name="wBst")
            nc.vector.memset(wBst[:], 0.0)

            # staging buffers (persistent, manual rotation; Tile serializes
            # WAR on reuse)
            rhsA = [cpool.tile([KA, TWA * NBF], F32, name=f"rhsA{i}")
                    for i in range(4)]
            rhsD = [cpool.tile([KD, TWD * NB], F32R, name=f"rhsD{i}")
                    for i in range(3)]

            def stage_A(w):
                eng = nc.sync if w % 2 == 0 else nc.scalar
                eng.dma_start(
                    rhsA[w % 4][:],
                    patA[:, w * TWA * NBF:(w + 1) * TWA * NBF],
                )

            def stage_D(w):
                nc.gpsimd.dma_start(
                    rhsD[w % 3][:],
                    patD[:, w * TWD * NB:(w + 1) * TWD * NB],
                )

            psA_tiles = {}
            xA_tiles = {}

            def conv_A(g):
                # conv for steps 4g..4g+3 (N=400), full fp32 for exactness;
                # two groups share one 2-bank PSUM tile (offsets 0 and 512)
                p = g // 2
                if g % 2 == 0:
                    psA_tiles[p] = psApool.tile([128, 1024], F32, name="psA")
                ps = psA_tiles[p]
                w = g // 4
                sl = (g % 4) * 4 * NBF
                nc.tensor.matmul(
                    ps[:, (g % 2) * 512:(g % 2) * 512 + 4 * NBF],
                    wA_sb[:],
                    rhsA[w % 4][:, sl:sl + 4 * NBF],
                    start=True,
                    stop=True,
                )

            def evict_A(p):
                # PSUM -> SBUF for steps 8p..8p+7, one ACT instruction
                ps = psA_tiles.pop(p)
                xA = xApool.tile([128, 8 * NBF], F32, name="xA")
                xA_tiles[p] = xA
                src_ap = bass.AP(
                    tensor=ps.tensor,
                    offset=ps.offset,
                    ap=[list(ps.ap[0]), [512, 2], [1, 4 * NBF]],
                )
                dst_ap = bass.AP(
                    tensor=xA.tensor,
                    offset=xA.offset,
                    ap=[list(xA.ap[0]), [4 * NBF, 2], [1, 4 * NBF]],
                )
                nc.scalar.activation(dst_ap, src_ap, AF.Identity,
                                     bias=0.0, scale=1.0)

            psB_tiles = {}

            def conv_B(k):
                # diff-conv for B-step k (N=800 as 2x400 in one 2-bank tile)
                w = k // TWD
                sl = (k % TWD) * NB
                ps = psBpool.tile([128, 1024], F32, name="psB")
                psB_tiles[k] = ps
                for h in range(2):
                    nc.tensor.matmul(
                        ps[:, h * 512:h * 512 + 400],
                        wD_sb[:],
                        rhsD[w % 3][:, sl + h * 400: sl + (h + 1) * 400],
                        start=True,
                        stop=True,
                    )

            vA_tiles = {}
            vB_tiles = {}

            def scan_A(t):
                blk = t // 8
                if t % 8 == 0:
                    vA_tiles[blk] = vApool.tile([128, 8 * NBF], F32, name="vA")
                vA = vA_tiles[blk]
                sl = (t % 8) * NBF
                xA = xA_tiles[t // 8]
                nc.vector.scalar_tensor_tensor(
                    out=vA[:, sl:sl + NBF],
                    in0=wAst[:],
                    scalar=alA[:],
                    in1=xA[:, sl:sl + NBF],
                    op0=AL.mult,
                    op1=AL.add,
                )
                nc.vector.scalar_tensor_tensor(
                    out=wAst[:],
                    in0=vA[:, sl:sl + NBF],
                    scalar=1.0,
                    in1=vA[:, sl:sl + NBF],
                    op0=AL.is_lt,
                    op1=AL.mult,
                )
                if t % 8 == 7:
                    xA_tiles.pop(blk, None)
                    eng = nc.sync if blk % 2 == 0 else nc.scalar
                    eng.dma_start(vAo[blk], vA[:])

            def scan_B(k):
                ps = psB_tiles.pop(k)
                xB = xBpool.tile([128, NB], F32, name="xB")
                src_ap = bass.AP(
                    tensor=ps.tensor,
                    offset=ps.offset,
                    ap=[list(ps.ap[0]), [512, 2], [1, 400]],
                )
                dst_ap = bass.AP(
                    tensor=xB.tensor,
                    offset=xB.offset,
                    ap=[list(xB.ap[0]), [400, 2], [1, 400]],
                )
                nc.scalar.activation(dst_ap, src_ap, AF.Abs,
                                     bias=0.0, scale=1.0)
                if k == LB:
                    # t=0 column block (c=0): temporal delta is defined as 0
                    nc.vector.memset(xB[:, 0:NBF], 0.0)
                vB = vBpool.tile([128, NB], F32, name="vB")
                vB_tiles[k] = vB
                nc.vector.scalar_tensor_tensor(
                    out=vB[:],
                    in0=wBst[:],
                    scalar=alB[:],
                    in1=xB[:],
                    op0=AL.mult,
                    op1=AL.add,
                )
                nc.vector.scalar_tensor_tensor(
                    out=wBst[:],
                    in0=vB[:],
                    scalar=1.0,
                    in1=vB[:],
                    op0=AL.is_lt,
                    op1=AL.mult,
                )
                if k >= LB:
                    eng = nc.scalar if k % 2 == 0 else nc.sync
                    eng.dma_start(vBo[k - LB], vB[:])

            # prologue: stage first windows, first convs + evicts
            stage_A(0)
            stage_D(0)
            stage_A(1)
            stage_D(1)
            stage_A(2)
            for g in range(4):
                conv_A(g)
            evict_A(0)
            conv_B(0)

            kB = 0
            for t in range(W):
                if t % TWA == 0 and t // TWA + 3 < W // TWA:
                    stage_A(t // TWA + 3)
                if t % 8 == 0:
                    p = t // 8
                    for g in (2 * p + 4, 2 * p + 5):
                        if g < 128:
                            conv_A(g)
                    if p + 1 < 64:
                        evict_A(p + 1)
                scan_A(t)
                k_target = ((t + 1) * SB) // W
                while kB < min(k_target, SB):
                    k = kB
                    if k % TWD == 0 and k // TWD + 2 < SB // TWD:
                        stage_D(k // TWD + 2)
                    if k + 1 < SB:
                        conv_B(k + 1)
                    scan_B(k)
                    kB += 1

    _split_multi_waits(nc)
    return nc


_NC_CACHE = [None]
LAST_RESULT = [None]


def _get_nc():
    if _NC_CACHE[0] is None:
        _NC_CACHE[0] = _build_nc()
    return _NC_CACHE[0]


def _host_prep(inputs, w3, b3, w7, b7):
    f32 = np.float32
    swv = np.lib.stride_tricks.sliding_window_view

    # weight matrices [K, 128]; cols 0:64 conv3 channels, 64:128 conv7
    wA = np.zeros((KA, 128), dtype=f32)
    w3r = w3.reshape(C, 3, 3)
    w7r = w7.reshape(C, 7, 7)
    for i in range(3):
        for j in range(3):
            wA[(i + 2) * 7 + (j + 2), 0:C] = w3r[:, i, j]
    for i in range(7):
        for j in range(7):
            wA[i * 7 + j, C:128] = w7r[:, i, j]
    wA[49, 0:C] = b3
    wA[49, C:128] = b7
    wD = (wA[:49] * f32(1.25)).astype(f32)   # threshold-normalized diff conv

    alphaA = np.full((128, 1), ALPHA[0], dtype=f32)
    alphaA[C:, 0] = ALPHA[1]
    alphaB = np.full((128, 1), ALPHA[2], dtype=f32)
    alphaB[C:, 0] = ALPHA[3]

    per_core = []
    for ci in range(NCORES):
        xc = inputs[ci * BL:(ci + 1) * BL]          # [4, 512, 25] (b, t, f)

        # A-side im2col: patA[(i*7+j), t*100 + f*4 + b] = xpad[b, t+j, f+i]
        xp = np.zeros((BL, W + 6, F + 6), dtype=f32)
        xp[:, 3:3 + W, 3:3 + F] = xc
        v = swv(xp, (7, 7), axis=(1, 2))            # [b, t, f, j, i]
        pA = np.ascontiguousarray(v.transpose(4, 3, 1, 2, 0))  # [i,j,t,f,b]
        patA = np.empty((KA, W * NBF), dtype=f32)
        patA[:49] = pA.reshape(49, W * NBF)
        patA[49] = 1.0                               # bias row

        # B-side: temporal diff on the padded domain.
        # xd[tau] = x[tau] - x[tau-1] for tau in [0, 512], xd[512] = -x[511]
        xdp = np.zeros((BL, LB + 3 + W + 3, F + 6), dtype=f32)
        xd = np.diff(xc, axis=1, prepend=np.zeros_like(xc[:, :1]))
        xdp[:, LB + 3:LB + 3 + W, 3:3 + F] = xd
        xdp[:, LB + 3 + W, 3:3 + F] = -xc[:, -1]
        vD = swv(xdp, (7, 7), axis=(1, 2))          # [b, w0, f, j, i]
        pD = np.ascontiguousarray(vD.transpose(4, 3, 1, 2, 0))  # [i,j,w0,f,b]
        pD = pD.reshape(49, LB + W - CH + CH, F, BL) if False else pD
        # columns: (k, c, f, b) with w0 = c*64 + k
        idx = (np.arange(PB)[None, :] * CH + np.arange(SB)[:, None])  # [k, c]
        patD = np.ascontiguousarray(
            pD.reshape(49, -1, F, BL)[:, idx.reshape(-1)]
        ).reshape(KD, SB * NB)

        per_core.append({
            "patA": patA,
            "patD": patD,
            "wA": wA,
            "wD": wD,
            "alphaA": alphaA,
            "alphaB": alphaB,
        })
    return per_core


def _host_post(res):
    f32 = np.float32
    outs = []
    for ci in range(NCORES):
        r = res.results[ci]
        # spikes = (v >= 1), matching the reference heaviside exactly
        sA = (np.asarray(r["vA"]) >= 1.0).astype(f32)   # [blk, ch, tl*100+n]
        sB = (np.asarray(r["vB"]) >= 1.0).astype(f32)   # [k, ch, c*100+n]
        sA = sA.reshape(W // 8, 128, 8, F, BL)
        sA = sA.transpose(1, 0, 2, 3, 4).reshape(128, W, F, BL)
        sB = sB.reshape(CH, 128, PB, F, BL)
        sB = sB.transpose(1, 2, 0, 3, 4).reshape(128, W, F, BL)
        out = (sA[:C] + sA[C:] + sB[:C] + sB[C:])   # [C, t, f, b]
        outs.append(out.transpose(3, 0, 2, 1))      # [b, C, f, t]
    return np.ascontiguousarray(np.concatenate(outs, axis=0), dtype=f32)


def kernel(inputs, w3, b3, w7, b7):
    nc = _get_nc()
    per_core = _host_prep(
        np.asarray(inputs, dtype=np.float32),
        np.asarray(w3, dtype=np.float32),
        np.asarray(b3, dtype=np.float32),
        np.asarray(w7, dtype=np.float32),
        np.asarray(b7, dtype=np.float32),
    )
    res = bass_utils.run_bass_kernel_spmd(
        nc, per_core, core_ids=list(range(NCORES))
    )
    LAST_RESULT[0] = res
    return _host_post(res)


# revision 12
# speedup vs baseline: 1.0236x; 1.0077x over previous
"""Trainium2 Bass kernel for nn_DynamicReceptiveEncoder (v2).

Structure (per core, 4 of 32 batch elements):
  PE  : conv3+conv7 as one K=50 matmul (bias via ones row, f32) feeding the
        A-side scan via PSUM; diff-conv (K=49, fp32r, x-diff done on host,
        1.25 threshold-normalization folded into weights) feeding B-side.
  GPS : exact 512-step LIF scan for the two raw-conv neurons (tau 20/50,
        long subthreshold memory - cannot be time-chunked), one
        scalar_tensor_tensor pair per step, reading conv PSUM directly.
  DVE : time-chunked LIF scan for the two |diff| neurons (tau 2/0.91,
        state forgets within ~10 steps): 8 chunks x (64+16) steps
        processed as 800 columns per instruction.
  ACT : |.| eviction of diff-conv PSUM; Sign(v-1) spike masks for both
        sides (bf16) which are DMA'd to DRAM.
  Host: im2col staging matrices (so device DMA is wide contiguous block
        copy), final spike summation across the four neuron masks.
"""

import sys

sys.path.insert(0, "/opt/trn_rl_repo")

import numpy as np

import concourse.bass as bass
import concourse.mybir as mybir
from concourse.tile import TileContext
from concourse import bass_utils

AL = mybir.AluOpType
AF = mybir.ActivationFunctionType
F32 = mybir.dt.float32
F32R = mybir.dt.float32r
BF16 = mybir.dt.bfloat16

# ---------------------------------------------------------------------------
# Patches for this walrus build (max ONE sync wait per instruction) and for
# the missing NTFF profile hook module.
# ---------------------------------------------------------------------------
import concourse.tile as _tile
from concourse.vector_clock import ScopedClock as _ScopedClock

_wsplit_counter = [0]


def _patched_drain_and_barrier(self, tick_clock, wait_clock):
    nc = self.nc
    drain_inst = nc.sync.drain()
    wait_clock.add_sem_waits(
        drain_inst.ins, _ScopedClock({None: tick_clock.global_clock})
    )
    si = drain_inst.ins.sync_info
    waits = list(si.on_wait) if si is not None else []
    if len(waits) > 1:
        updates = list(si.on_update) if si is not None else []
        drain_inst.ins.sync_info = mybir.SyncInfo(on_wait=[], on_update=updates)
        for w in waits:
            nop_inst = nc.sync.nop(nofuse=True)
            nop_inst.ins.sync_info = mybir.SyncInfo(on_wait=[w], on_update=[])

    nc.all_engine_barrier()
    assert self.sems is not None
    popped = nc._tile_sem_poison_stack.pop()
    assert popped is self._sem_poison
    nc.clear_and_free_semaphores(list(self.sems.allocated().values()))
    nc.all_engine_barrier()


_tile.TileContext._drain_and_barrier = _patched_drain_and_barrier


def _split_multi_waits(nc, max_waits=1):
    for f in nc.m.functions:
        for bb in f.blocks:
            insts = bb.instructions
            i = 0
            while i < len(insts):
                inst = insts[i]
                si = inst.sync_info
                if si is not None and len(si.on_wait) > max_waits:
                    waits = list(si.on_wait)
                    extra, keep = waits[:-max_waits], waits[-max_waits:]
                    inst.sync_info = mybir.SyncInfo(
                        on_wait=keep, on_update=list(si.on_update)
                    )
                    for w in extra:
                        _wsplit_counter[0] += 1
                        nop = mybir.InstNoOp(
                            name=f"wsplit_{_wsplit_counter[0]}", ins=[], outs=[]
                        )
                        nop.engine = inst.engine
                        nop.sync_info = mybir.SyncInfo(on_wait=[w], on_update=[])
                        insts.insert(i, nop)
                        i += 1
                i += 1


def _install_ntff_hook():
    import contextlib, ctypes, types

    try:
        lib = ctypes.CDLL("/opt/axon/libaxon_pjrt.so")
    except OSError:
        return
    if not hasattr(lib, "axon_start_nrt_profile"):
        return
    lib.axon_start_nrt_profile.argtypes = [
        ctypes.POINTER(ctypes.c_int64),
        ctypes.c_size_t,
    ]
    lib.axon_start_nrt_profile.restype = ctypes.c_int64
    lib.axon_stop_nrt_profile.argtypes = [ctypes.c_char_p]
    lib.axon_stop_nrt_profile.restype = ctypes.c_int64

    @contextlib.contextmanager
    def _hook(output_dir, device_ids):
        import jax

        jax.devices()
        if device_ids:
            ids = (ctypes.c_int64 * len(device_ids))(*device_ids)
            rc = lib.axon_start_nrt_profile(ids, len(device_ids))
        else:
            rc = lib.axon_start_nrt_profile(None, 0)
        if rc != 0:
            raise RuntimeError(f"axon_start_nrt_profile rc={rc}")
        try:
            yield
        finally:
            lib.axon_stop_nrt_profile(str(output_dir).encode())

    mod = types.ModuleType("antenv.axon_hooks")
    holder = [_hook]
    mod.set_axon_ntff_profile_hook = lambda h: holder.__setitem__(0, h)
    mod.get_axon_ntff_profile_hook = lambda: holder[0]
    sys.modules["antenv.axon_hooks"] = mod
    try:
        import antenv

        antenv.axon_hooks = mod
    except ImportError:
        pass


_install_ntff_hook()

# ---------------------------------------------------------------------------
# Problem constants
# ---------------------------------------------------------------------------
B, W, F, C = 32, 512, 25, 64
NCORES = 8
BL = B // NCORES            # 4 batch elements per core
NBF = BL * F                # 100 (f, b) columns
KA = 50                     # 49 taps + bias/ones row
KD = 49                     # diff-conv taps only (bias cancels)

PB = 8                      # B-side time chunks
CH = W // PB                # 64 steps per chunk
LB = 16                     # B-side warmup steps
SB = CH + LB                # 80 sequential B steps
NB = PB * NBF               # 800 B-side columns per step

TWA = 16                    # A-side staging window (steps)
TWD = 8                     # B-side staging window (B-steps, SB=88 -> 11 windows)

TAU = (20.0, 50.0, 2.0, 0.91)
ALPHA = tuple(np.float32(1.0 - 1.0 / t) for t in TAU)


def _build_nc():
    nc = bass.Bass()
    patA = nc.dram_tensor("patA", [KA, W * NBF], F32, kind="ExternalInput")
    patD = nc.dram_tensor("patD", [KD, SB * NB], F32R, kind="ExternalInput")
    wAt = nc.dram_tensor("wA", [KA, 128], F32, kind="ExternalInput")
    wDt = nc.dram_tensor("wD", [KD, 128], F32R, kind="ExternalInput")
    alAt = nc.dram_tensor("alphaA", [128, 1], F32, kind="ExternalInput")
    alBt = nc.dram_tensor("alphaB", [128, 1], F32, kind="ExternalInput")
    vAo = nc.dram_tensor("vA", [W // 8, 128, 8 * NBF], F32, kind="ExternalOutput")
    vBo = nc.dram_tensor("vB", [CH, 128, NB], F32, kind="ExternalOutput")

    with TileContext(nc) as tc:
        with (
            tc.tile_pool(name="consts", bufs=1) as cpool,
            tc.tile_pool(name="vA", bufs=3) as vApool,
            tc.tile_pool(name="vB", bufs=3) as vBpool,
            tc.tile_pool(name="xA", bufs=3) as xApool,
            tc.tile_pool(name="xB", bufs=3) as xBpool,
            tc.tile_pool(name="psA", bufs=2, space="PSUM") as psApool,
            tc.tile_pool(name="psB", bufs=2, space="PSUM") as psBpool,
        ):
            wA_sb = cpool.tile([KA, 128], F32, name="wA_sb")
            nc.sync.dma_start(wA_sb[:], wAt[:])
            wD_sb = cpool.tile([KD, 128], F32R, name="wD_sb")
            nc.sync.dma_start(wD_sb[:], wDt[:])
            alA = cpool.tile([128, 1], F32, name="alA")
            nc.sync.dma_start(alA[:], alAt[:])
            alB = cpool.tile([128, 1], F32, name="alB")
            nc.sync.dma_start(alB[:], alBt[:])
            wAst = cpool.tile([128, NBF], F32, name="wAst")
            nc.vector.memset(wAst[:], 0.0)
            wBst = cpool.tile([128, NB], F32, name="wBst")
            nc.vector.memset(wBst[:], 0.0)

            # staging buffers (persistent, manual rotation; Tile serializes
            # WAR on reuse)
            rhsA = [cpool.tile([KA, TWA * NBF], F32, name=f"rhsA{i}")
                    for i in range(3)]
            rhsD = [cpool.tile([KD, TWD * NB], F32R, name=f"rhsD{i}")
                    for i in range(3)]

            def stage_A(w):
                eng = nc.sync if w % 2 == 0 else nc.scalar
                eng.dma_start(
                    rhsA[w % 3][:],
                    patA[:, w * TWA * NBF:(w + 1) * TWA * NBF],
                )

            def stage_D(w):
                nc.gpsimd.dma_start(
                    rhsD[w % 3][:],
                    patD[:, w * TWD * NB:(w + 1) * TWD * NB],
                )

            psA_tiles = {}
            xA_tiles = {}

            def conv_A(g):
                # conv for steps 4g..4g+3 (N=400), full fp32 for exactness;
                # two groups share one 2-bank PSUM tile (offsets 0 and 512)
                p = g // 2
                if g % 2 == 0:
                    psA_tiles[p] = psApool.tile([128, 1024], F32, name="psA")
                ps = psA_tiles[p]
                w = g // 4
                sl = (g % 4) * 4 * NBF
                nc.tensor.matmul(
                    ps[:, (g % 2) * 512:(g % 2) * 512 + 4 * NBF],
                    wA_sb[:],
                    rhsA[w % 3][:, sl:sl + 4 * NBF],
                    start=True,
                    stop=True,
                )

            def evict_A(p):
                # PSUM -> SBUF for steps 8p..8p+7, one ACT instruction
                ps = psA_tiles.pop(p)
                xA = xApool.tile([128, 8 * NBF], F32, name="xA")
                xA_tiles[p] = xA
                src_ap = bass.AP(
                    tensor=ps.tensor,
                    offset=ps.offset,
                    ap=[list(ps.ap[0]), [512, 2], [1, 4 * NBF]],
                )
                dst_ap = bass.AP(
                    tensor=xA.tensor,
                    offset=xA.offset,
                    ap=[list(xA.ap[0]), [4 * NBF, 2], [1, 4 * NBF]],
                )
                nc.scalar.activation(dst_ap, src_ap, AF.Identity,
                                     bias=0.0, scale=1.0)

            psB_tiles = {}

            def conv_B(k):
                # diff-conv for B-step k (N=800 as 2x400 in one 2-bank tile)
                w = k // TWD
                sl = (k % TWD) * NB
                ps = psBpool.tile([128, 1024], F32, name="psB")
                psB_tiles[k] = ps
                for h in range(2):
                    nc.tensor.matmul(
                        ps[:, h * 512:h * 512 + 400],
                        wD_sb[:],
                        rhsD[w % 3][:, sl + h * 400: sl + (h + 1) * 400],
                        start=True,
                        stop=True,
                    )

            vA_tiles = {}
            vB_tiles = {}

            def scan_A(t):
                blk = t // 8
                if t % 8 == 0:
                    vA_tiles[blk] = vApool.tile([128, 8 * NBF], F32, name="vA")
                vA = vA_tiles[blk]
                sl = (t % 8) * NBF
                xA = xA_tiles[t // 8]
                nc.vector.scalar_tensor_tensor(
                    out=vA[:, sl:sl + NBF],
                    in0=wAst[:],
                    scalar=alA[:],
                    in1=xA[:, sl:sl + NBF],
                    op0=AL.mult,
                    op1=AL.add,
                )
                nc.vector.scalar_tensor_tensor(
                    out=wAst[:],
                    in0=vA[:, sl:sl + NBF],
                    scalar=1.0,
                    in1=vA[:, sl:sl + NBF],
                    op0=AL.is_lt,
                    op1=AL.mult,
                )
                if t % 8 == 7:
                    xA_tiles.pop(blk, None)
                    eng = nc.sync if blk % 2 == 0 else nc.scalar
                    eng.dma_start(vAo[blk], vA[:])

            def scan_B(k):
                ps = psB_tiles.pop(k)
                xB = xBpool.tile([128, NB], F32, name="xB")
                src_ap = bass.AP(
                    tensor=ps.tensor,
                    offset=ps.offset,
                    ap=[list(ps.ap[0]), [512, 2], [1, 400]],
                )
                dst_ap = bass.AP(
                    tensor=xB.tensor,
                    offset=xB.offset,
                    ap=[list(xB.ap[0]), [400, 2], [1, 400]],
                )
                nc.scalar.activation(dst_ap, src_ap, AF.Abs,
                                     bias=0.0, scale=1.0)
                if k == LB:
                    # t=0 column block (c=0): temporal delta is defined as 0
                    nc.vector.memset(xB[:, 0:NBF], 0.0)
                vB = vBpool.tile([128, NB], F32, name="vB")
                vB_tiles[k] = vB
                nc.vector.scalar_tensor_tensor(
                    out=vB[:],
                    in0=wBst[:],
                    scalar=alB[:],
                    in1=xB[:],
                    op0=AL.mult,
                    op1=AL.add,
                )
                nc.vector.scalar_tensor_tensor(
                    out=wBst[:],
                    in0=vB[:],
                    scalar=1.0,
                    in1=vB[:],
                    op0=AL.is_lt,
                    op1=AL.mult,
                )
                if k >= LB:
                    eng = nc.scalar if k % 2 == 0 else nc.sync
                    eng.dma_start(vBo[k - LB], vB[:])

            # prologue: stage first windows, first convs + evicts
            stage_A(0)
            stage_D(0)
            stage_A(1)
            stage_D(1)
            for g in range(4):
                conv_A(g)
            evict_A(0)
            conv_B(0)

            kB = 0
            for t in range(W):
                if t % TWA == 0 and t // TWA + 2 < W // TWA:
                    stage_A(t // TWA + 2)
                if t % 8 == 0:
                    p = t // 8
                    for g in (2 * p + 4, 2 * p + 5):
                        if g < 128:
                            conv_A(g)
                    if p + 1 < 64:
                        evict_A(p + 1)
                scan_A(t)
                k_target = ((t + 1) * SB) // W
                while kB < min(k_target, SB):
                    k = kB
                    if k % TWD == 0 and k // TWD + 2 < SB // TWD:
                        stage_D(k // TWD + 2)
                    if k + 1 < SB:
                        conv_B(k + 1)
                    scan_B(k)
                    kB += 1

    _split_multi_waits(nc)
    return nc


_NC_CACHE = [None]
LAST_RESULT = [None]


def _get_nc():
    if _NC_CACHE[0] is None:
        _NC_CACHE[0] = _build_nc()
    return _NC_CACHE[0]


def _host_prep(inputs, w3, b3, w7, b7):
    f32 = np.float32
    swv = np.lib.stride_tricks.sliding_window_view

    # weight matrices [K, 128]; cols 0:64 conv3 channels, 64:128 conv7
    wA = np.zeros((KA, 128), dtype=f32)
    w3r = w3.reshape(C, 3, 3)
    w7r = w7.reshape(C, 7, 7)
    for i in range(3):
        for j in range(3):
            wA[(i + 2) * 7 + (j + 2), 0:C] = w3r[:, i, j]
    for i in range(7):
        for j in range(7):
            wA[i * 7 + j, C:128] = w7r[:, i, j]
    wA[49, 0:C] = b3
    wA[49, C:128] = b7
    wD = (wA[:49] * f32(1.25)).astype(f32)   # threshold-normalized diff conv

    alphaA = np.full((128, 1), ALPHA[0], dtype=f32)
    alphaA[C:, 0] = ALPHA[1]
    alphaB = np.full((128, 1), ALPHA[2], dtype=f32)
    alphaB[C:, 0] = ALPHA[3]

    per_core = []
    for ci in range(NCORES):
        xc = inputs[ci * BL:(ci + 1) * BL]          # [4, 512, 25] (b, t, f)

        # A-side im2col: patA[(i*7+j), t*100 + f*4 + b] = xpad[b, t+j, f+i]
        xp = np.zeros((BL, W + 6, F + 6), dtype=f32)
        xp[:, 3:3 + W, 3:3 + F] = xc
        v = swv(xp, (7, 7), axis=(1, 2))            # [b, t, f, j, i]
        pA = np.ascontiguousarray(v.transpose(4, 3, 1, 2, 0))  # [i,j,t,f,b]
        patA = np.empty((KA, W * NBF), dtype=f32)
        patA[:49] = pA.reshape(49, W * NBF)
        patA[49] = 1.0                               # bias row

        # B-side: temporal diff on the padded domain.
        # xd[tau] = x[tau] - x[tau-1] for tau in [0, 512], xd[512] = -x[511]
        xdp = np.zeros((BL, LB + 3 + W + 3, F + 6), dtype=f32)
        xd = np.diff(xc, axis=1, prepend=np.zeros_like(xc[:, :1]))
        xdp[:, LB + 3:LB + 3 + W, 3:3 + F] = xd
        xdp[:, LB + 3 + W, 3:3 + F] = -xc[:, -1]
        vD = swv(xdp, (7, 7), axis=(1, 2))          # [b, w0, f, j, i]
        pD = np.ascontiguousarray(vD.transpose(4, 3, 1, 2, 0))  # [i,j,w0,f,b]
        pD = pD.reshape(49, LB + W - CH + CH, F, BL) if False else pD
        # columns: (k, c, f, b) with w0 = c*64 + k
        idx = (np.arange(PB)[None, :] * CH + np.arange(SB)[:, None])  # [k, c]
        patD = np.ascontiguousarray(
            pD.reshape(49, -1, F, BL)[:, idx.reshape(-1)]
        ).reshape(KD, SB * NB)

        per_core.append({
            "patA": patA,
            "patD": patD,
            "wA": wA,
            "wD": wD,
            "alphaA": alphaA,
            "alphaB": alphaB,
        })
    return per_core


def _host_post(res):
    f32 = np.float32
    outs = []
    for ci in range(NCORES):
        r = res.results[ci]
        # spikes = (v >= 1), matching the reference heaviside exactly
        sA = (np.asarray(r["vA"]) >= 1.0).astype(f32)   # [blk, ch, tl*100+n]
        sB = (np.asarray(r["vB"]) >= 1.0).astype(f32)   # [k, ch, c*100+n]
        sA = sA.reshape(W // 8, 128, 8, F, BL)
        sA = sA.transpose(1, 0, 2, 3, 4).reshape(128, W, F, BL)
        sB = sB.reshape(CH, 128, PB, F, BL)
        sB = sB.transpose(1, 2, 0, 3, 4).reshape(128, W, F, BL)
        out = (sA[:C] + sA[C:] + sB[:C] + sB[C:])   # [C, t, f, b]
        outs.append(out.transpose(3, 0, 2, 1))      # [b, C, f, t]
    return np.ascontiguousarray(np.concatenate(outs, axis=0), dtype=f32)


def kernel(inputs, w3, b3, w7, b7):
    nc = _get_nc()
    per_core = _host_prep(
        np.asarray(inputs, dtype=np.float32),
        np.asarray(w3, dtype=np.float32),
        np.asarray(b3, dtype=np.float32),
        np.asarray(w7, dtype=np.float32),
        np.asarray(b7, dtype=np.float32),
    )
    res = bass_utils.run_bass_kernel_spmd(
        nc, per_core, core_ids=list(range(NCORES))
    )
    LAST_RESULT[0] = res
    return _host_post(res)


# revision 13
# speedup vs baseline: 1.0505x; 1.0263x over previous
"""Trainium2 Bass kernel for nn_DynamicReceptiveEncoder (v2).

Structure (per core, 4 of 32 batch elements):
  PE  : conv3+conv7 as one K=50 matmul (bias via ones row, f32) feeding the
        A-side scan via PSUM; diff-conv (K=49, fp32r, x-diff done on host,
        1.25 threshold-normalization folded into weights) feeding B-side.
  GPS : exact 512-step LIF scan for the two raw-conv neurons (tau 20/50,
        long subthreshold memory - cannot be time-chunked), one
        scalar_tensor_tensor pair per step, reading conv PSUM directly.
  DVE : time-chunked LIF scan for the two |diff| neurons (tau 2/0.91,
        state forgets within ~10 steps): 8 chunks x (64+16) steps
        processed as 800 columns per instruction.
  ACT : |.| eviction of diff-conv PSUM; Sign(v-1) spike masks for both
        sides (bf16) which are DMA'd to DRAM.
  Host: im2col staging matrices (so device DMA is wide contiguous block
        copy), final spike summation across the four neuron masks.
"""

import sys

sys.path.insert(0, "/opt/trn_rl_repo")

import numpy as np

import concourse.bass as bass
import concourse.mybir as mybir
from concourse.tile import TileContext
from concourse import bass_utils

AL = mybir.AluOpType
AF = mybir.ActivationFunctionType
F32 = mybir.dt.float32
F32R = mybir.dt.float32r
BF16 = mybir.dt.bfloat16

# ---------------------------------------------------------------------------
# Patches for this walrus build (max ONE sync wait per instruction) and for
# the missing NTFF profile hook module.
# ---------------------------------------------------------------------------
import concourse.tile as _tile
from concourse.vector_clock import ScopedClock as _ScopedClock

_wsplit_counter = [0]


def _patched_drain_and_barrier(self, tick_clock, wait_clock):
    nc = self.nc
    drain_inst = nc.sync.drain()
    wait_clock.add_sem_waits(
        drain_inst.ins, _ScopedClock({None: tick_clock.global_clock})
    )
    si = drain_inst.ins.sync_info
    waits = list(si.on_wait) if si is not None else []
    if len(waits) > 1:
        updates = list(si.on_update) if si is not None else []
        drain_inst.ins.sync_info = mybir.SyncInfo(on_wait=[], on_update=updates)
        for w in waits:
            nop_inst = nc.sync.nop(nofuse=True)
            nop_inst.ins.sync_info = mybir.SyncInfo(on_wait=[w], on_update=[])

    nc.all_engine_barrier()
    assert self.sems is not None
    popped = nc._tile_sem_poison_stack.pop()
    assert popped is self._sem_poison
    nc.clear_and_free_semaphores(list(self.sems.allocated().values()))
    nc.all_engine_barrier()


_tile.TileContext._drain_and_barrier = _patched_drain_and_barrier


def _split_multi_waits(nc, max_waits=1):
    for f in nc.m.functions:
        for bb in f.blocks:
            insts = bb.instructions
            i = 0
            while i < len(insts):
                inst = insts[i]
                si = inst.sync_info
                if si is not None and len(si.on_wait) > max_waits:
                    waits = list(si.on_wait)
                    extra, keep = waits[:-max_waits], waits[-max_waits:]
                    inst.sync_info = mybir.SyncInfo(
                        on_wait=keep, on_update=list(si.on_update)
                    )
                    for w in extra:
                        _wsplit_counter[0] += 1
                        nop = mybir.InstNoOp(
                            name=f"wsplit_{_wsplit_counter[0]}", ins=[], outs=[]
                        )
                        nop.engine = inst.engine
                        nop.sync_info = mybir.SyncInfo(on_wait=[w], on_update=[])
                        insts.insert(i, nop)
                        i += 1
                i += 1


def _install_ntff_hook():
    import contextlib, ctypes, types

    try:
        lib = ctypes.CDLL("/opt/axon/libaxon_pjrt.so")
    except OSError:
        return
    if not hasattr(lib, "axon_start_nrt_profile"):
        return
    lib.axon_start_nrt_profile.argtypes = [
        ctypes.POINTER(ctypes.c_int64),
        ctypes.c_size_t,
    ]
    lib.axon_start_nrt_profile.restype = ctypes.c_int64
    lib.axon_stop_nrt_profile.argtypes = [ctypes.c_char_p]
    lib.axon_stop_nrt_profile.restype = ctypes.c_int64

    @contextlib.contextmanager
    def _hook(output_dir, device_ids):
        import jax

        jax.devices()
        if device_ids:
            ids = (ctypes.c_int64 * len(device_ids))(*device_ids)
            rc = lib.axon_start_nrt_profile(ids, len(device_ids))
        else:
            rc = lib.axon_start_nrt_profile(None, 0)
        if rc != 0:
            raise RuntimeError(f"axon_start_nrt_profile rc={rc}")
        try:
            yield
        finally:
            lib.axon_stop_nrt_profile(str(output_dir).encode())

    mod = types.ModuleType("antenv.axon_hooks")
    holder = [_hook]
    mod.set_axon_ntff_profile_hook = lambda h: holder.__setitem__(0, h)
    mod.get_axon_ntff_profile_hook = lambda: holder[0]
    sys.modules["antenv.axon_hooks"] = mod
    try:
        import antenv

        antenv.axon_hooks = mod
    except ImportError:
        pass


_install_ntff_hook()

# ---------------------------------------------------------------------------
# Problem constants
# ---------------------------------------------------------------------------
B, W, F, C = 32, 512, 25, 64
NCORES = 8
BL = B // NCORES            # 4 batch elements per core
NBF = BL * F                # 100 (f, b) columns
KA = 50                     # 49 taps + bias/ones row
KD = 49                     # diff-conv taps only (bias cancels)

PB = 8                      # B-side time chunks
CH = W // PB                # 64 steps per chunk
LB = 16                     # B-side warmup steps
SB = CH + LB                # 80 sequential B steps
NB = PB * NBF               # 800 B-side columns per step

TWA = 16                    # A-side staging window (steps)
TWD = 8                     # B-side staging window (B-steps, SB=88 -> 11 windows)

TAU = (20.0, 50.0, 2.0, 0.91)
ALPHA = tuple(np.float32(1.0 - 1.0 / t) for t in TAU)


def _build_nc():
    nc = bass.Bass()
    patA = nc.dram_tensor("patA", [KA, W * NBF], F32, kind="ExternalInput")
    patD = nc.dram_tensor("patD", [KD, SB * NB], F32R, kind="ExternalInput")
    wAt = nc.dram_tensor("wA", [KA, 128], F32, kind="ExternalInput")
    wDt = nc.dram_tensor("wD", [KD, 128], F32R, kind="ExternalInput")
    alAt = nc.dram_tensor("alphaA", [128, 1], F32, kind="ExternalInput")
    alBt = nc.dram_tensor("alphaB", [128, 1], F32, kind="ExternalInput")
    vAo = nc.dram_tensor("vA", [W // 8, 128, 8 * NBF], F32, kind="ExternalOutput")
    vBo = nc.dram_tensor("vB", [CH, 128, NB], F32, kind="ExternalOutput")

    with TileContext(nc) as tc:
        with (
            tc.tile_pool(name="consts", bufs=1) as cpool,
            tc.tile_pool(name="vA", bufs=3) as vApool,
            tc.tile_pool(name="vB", bufs=3) as vBpool,
            tc.tile_pool(name="xA", bufs=3) as xApool,
            tc.tile_pool(name="xB", bufs=3) as xBpool,
            tc.tile_pool(name="psA", bufs=2, space="PSUM") as psApool,
            tc.tile_pool(name="psB", bufs=2, space="PSUM") as psBpool,
        ):
            wA_sb = cpool.tile([KA, 128], F32, name="wA_sb")
            nc.sync.dma_start(wA_sb[:], wAt[:])
            wD_sb = cpool.tile([KD, 128], F32R, name="wD_sb")
            nc.sync.dma_start(wD_sb[:], wDt[:])
            alA = cpool.tile([128, 1], F32, name="alA")
            nc.sync.dma_start(alA[:], alAt[:])
            alB = cpool.tile([128, 1], F32, name="alB")
            nc.sync.dma_start(alB[:], alBt[:])
            wAst = cpool.tile([128, NBF], F32, name="wAst")
            nc.vector.memset(wAst[:], 0.0)
            wBst = cpool.tile([128, NB], F32, name="wBst")
            nc.vector.memset(wBst[:], 0.0)

            # staging buffers (persistent, manual rotation; Tile serializes
            # WAR on reuse)
            rhsA = [cpool.tile([KA, TWA * NBF], F32, name=f"rhsA{i}")
                    for i in range(3)]
            rhsD = [cpool.tile([KD, TWD * NB], F32R, name=f"rhsD{i}")
                    for i in range(3)]

            def stage_A(w):
                eng = nc.sync if w % 2 == 0 else nc.scalar
                eng.dma_start(
                    rhsA[w % 3][:],
                    patA[:, w * TWA * NBF:(w + 1) * TWA * NBF],
                )

            def stage_D(w):
                nc.gpsimd.dma_start(
                    rhsD[w % 3][:],
                    patD[:, w * TWD * NB:(w + 1) * TWD * NB],
                )

            psA_tiles = {}
            xA_tiles = {}

            def conv_A(g):
                # conv for steps 4g..4g+3 (N=400), full fp32 for exactness;
                # two groups share one 2-bank PSUM tile (offsets 0 and 512)
                p = g // 2
                if g % 2 == 0:
                    psA_tiles[p] = psApool.tile([128, 1024], F32, name="psA")
                ps = psA_tiles[p]
                w = g // 4
                sl = (g % 4) * 4 * NBF
                nc.tensor.matmul(
                    ps[:, (g % 2) * 512:(g % 2) * 512 + 4 * NBF],
                    wA_sb[:],
                    rhsA[w % 3][:, sl:sl + 4 * NBF],
                    start=True,
                    stop=True,
                )

            def evict_A(p):
                # PSUM -> SBUF for steps 8p..8p+7, one ACT instruction
                ps = psA_tiles.pop(p)
                xA = xApool.tile([128, 8 * NBF], F32, name="xA")
                xA_tiles[p] = xA
                src_ap = bass.AP(
                    tensor=ps.tensor,
                    offset=ps.offset,
                    ap=[list(ps.ap[0]), [512, 2], [1, 4 * NBF]],
                )
                dst_ap = bass.AP(
                    tensor=xA.tensor,
                    offset=xA.offset,
                    ap=[list(xA.ap[0]), [4 * NBF, 2], [1, 4 * NBF]],
                )
                nc.scalar.activation(dst_ap, src_ap, AF.Identity,
                                     bias=0.0, scale=1.0)

            psB_tiles = {}

            def conv_B(k):
                # diff-conv for B-step k (N=800 as 2x400 in one 2-bank tile)
                w = k // TWD
                sl = (k % TWD) * NB
                ps = psBpool.tile([128, 1024], F32, name="psB")
                psB_tiles[k] = ps
                for h in range(2):
                    nc.tensor.matmul(
                        ps[:, h * 512:h * 512 + 400],
                        wD_sb[:],
                        rhsD[w % 3][:, sl + h * 400: sl + (h + 1) * 400],
                        start=True,
                        stop=True,
                    )

            vA_tiles = {}
            vB_tiles = {}

            def scan_A(t):
                blk = t // 8
                if t % 8 == 0:
                    vA_tiles[blk] = vApool.tile([128, 8 * NBF], F32, name="vA")
                vA = vA_tiles[blk]
                sl = (t % 8) * NBF
                xA = xA_tiles[t // 8]
                nc.vector.scalar_tensor_tensor(
                    out=vA[:, sl:sl + NBF],
                    in0=wAst[:],
                    scalar=alA[:],
                    in1=xA[:, sl:sl + NBF],
                    op0=AL.mult,
                    op1=AL.add,
                )
                nc.vector.scalar_tensor_tensor(
                    out=wAst[:],
                    in0=vA[:, sl:sl + NBF],
                    scalar=1.0,
                    in1=vA[:, sl:sl + NBF],
                    op0=AL.is_lt,
                    op1=AL.mult,
                )
                if t % 8 == 7:
                    xA_tiles.pop(blk, None)
                    eng = nc.sync if blk % 2 == 0 else nc.scalar
                    eng.dma_start(vAo[blk], vA[:])

            pendingB = []

            def scan_B(k):
                # evict runs now (ACT); the 4 DVE scan pieces are queued and
                # dripped one-per-A-step to avoid head-of-line blocking of
                # the serial A chain on the in-order DVE queue.
                ps = psB_tiles.pop(k)
                xB = xBpool.tile([128, NB], F32, name="xB")
                src_ap = bass.AP(
                    tensor=ps.tensor,
                    offset=ps.offset,
                    ap=[list(ps.ap[0]), [512, 2], [1, 400]],
                )
                dst_ap = bass.AP(
                    tensor=xB.tensor,
                    offset=xB.offset,
                    ap=[list(xB.ap[0]), [400, 2], [1, 400]],
                )
                nc.scalar.activation(dst_ap, src_ap, AF.Abs,
                                     bias=0.0, scale=1.0)
                vB = vBpool.tile([128, NB], F32, name="vB")
                vB_tiles[k] = vB
                H = NB // 2

                def vpiece(h):
                    def go():
                        if h == 0 and k == LB:
                            nc.vector.memset(xB[:, 0:NBF], 0.0)
                        nc.vector.scalar_tensor_tensor(
                            out=vB[:, h * H:(h + 1) * H],
                            in0=wBst[:, h * H:(h + 1) * H],
                            scalar=alB[:],
                            in1=xB[:, h * H:(h + 1) * H],
                            op0=AL.mult,
                            op1=AL.add,
                        )
                    return go

                def wpiece(h):
                    def go():
                        nc.vector.scalar_tensor_tensor(
                            out=wBst[:, h * H:(h + 1) * H],
                            in0=vB[:, h * H:(h + 1) * H],
                            scalar=1.0,
                            in1=vB[:, h * H:(h + 1) * H],
                            op0=AL.is_lt,
                            op1=AL.mult,
                        )
                        if h == 1 and k >= LB:
                            eng = nc.scalar if k % 2 == 0 else nc.sync
                            eng.dma_start(vBo[k - LB], vB[:])
                    return go

                pendingB.extend([vpiece(0), vpiece(1), wpiece(0), wpiece(1)])

            # prologue: stage first windows, first convs + evicts
            stage_A(0)
            stage_D(0)
            stage_A(1)
            stage_D(1)
            for g in range(4):
                conv_A(g)
            evict_A(0)
            conv_B(0)

            kB = 0
            for t in range(W):
                if t % TWA == 0 and t // TWA + 2 < W // TWA:
                    stage_A(t // TWA + 2)
                if t % 8 == 0:
                    p = t // 8
                    for g in (2 * p + 4, 2 * p + 5):
                        if g < 128:
                            conv_A(g)
                    if p + 1 < 64:
                        evict_A(p + 1)
                scan_A(t)
                k_target = ((t + 1) * SB) // W
                while kB < min(k_target, SB):
                    k = kB
                    if k % TWD == 0 and k // TWD + 2 < SB // TWD:
                        stage_D(k // TWD + 2)
                    if k + 1 < SB:
                        conv_B(k + 1)
                    scan_B(k)
                    kB += 1
                if pendingB:
                    pendingB.pop(0)()
            while pendingB:
                pendingB.pop(0)()

    _split_multi_waits(nc)
    return nc


_NC_CACHE = [None]
LAST_RESULT = [None]


def _get_nc():
    if _NC_CACHE[0] is None:
        _NC_CACHE[0] = _build_nc()
    return _NC_CACHE[0]


def _host_prep(inputs, w3, b3, w7, b7):
    f32 = np.float32
    swv = np.lib.stride_tricks.sliding_window_view

    # weight matrices [K, 128]; cols 0:64 conv3 channels, 64:128 conv7
    wA = np.zeros((KA, 128), dtype=f32)
    w3r = w3.reshape(C, 3, 3)
    w7r = w7.reshape(C, 7, 7)
    for i in range(3):
        for j in range(3):
            wA[(i + 2) * 7 + (j + 2), 0:C] = w3r[:, i, j]
    for i in range(7):
        for j in range(7):
            wA[i * 7 + j, C:128] = w7r[:, i, j]
    wA[49, 0:C] = b3
    wA[49, C:128] = b7
    wD = (wA[:49] * f32(1.25)).astype(f32)   # threshold-normalized diff conv

    alphaA = np.full((128, 1), ALPHA[0], dtype=f32)
    alphaA[C:, 0] = ALPHA[1]
    alphaB = np.full((128, 1), ALPHA[2], dtype=f32)
    alphaB[C:, 0] = ALPHA[3]

    per_core = []
    for ci in range(NCORES):
        xc = inputs[ci * BL:(ci + 1) * BL]          # [4, 512, 25] (b, t, f)

        # A-side im2col: patA[(i*7+j), t*100 + f*4 + b] = xpad[b, t+j, f+i]
        xp = np.zeros((BL, W + 6, F + 6), dtype=f32)
        xp[:, 3:3 + W, 3:3 + F] = xc
        v = swv(xp, (7, 7), axis=(1, 2))            # [b, t, f, j, i]
        pA = np.ascontiguousarray(v.transpose(4, 3, 1, 2, 0))  # [i,j,t,f,b]
        patA = np.empty((KA, W * NBF), dtype=f32)
        patA[:49] = pA.reshape(49, W * NBF)
        patA[49] = 1.0                               # bias row

        # B-side: temporal diff on the padded domain.
        # xd[tau] = x[tau] - x[tau-1] for tau in [0, 512], xd[512] = -x[511]
        xdp = np.zeros((BL, LB + 3 + W + 3, F + 6), dtype=f32)
        xd = np.diff(xc, axis=1, prepend=np.zeros_like(xc[:, :1]))
        xdp[:, LB + 3:LB + 3 + W, 3:3 + F] = xd
        xdp[:, LB + 3 + W, 3:3 + F] = -xc[:, -1]
        vD = swv(xdp, (7, 7), axis=(1, 2))          # [b, w0, f, j, i]
        pD = np.ascontiguousarray(vD.transpose(4, 3, 1, 2, 0))  # [i,j,w0,f,b]
        pD = pD.reshape(49, LB + W - CH + CH, F, BL) if False else pD
        # columns: (k, c, f, b) with w0 = c*64 + k
        idx = (np.arange(PB)[None, :] * CH + np.arange(SB)[:, None])  # [k, c]
        patD = np.ascontiguousarray(
            pD.reshape(49, -1, F, BL)[:, idx.reshape(-1)]
        ).reshape(KD, SB * NB)

        per_core.append({
            "patA": patA,
            "patD": patD,
            "wA": wA,
            "wD": wD,
            "alphaA": alphaA,
            "alphaB": alphaB,
        })
    return per_core


def _host_post(res):
    f32 = np.float32
    outs = []
    for ci in range(NCORES):
        r = res.results[ci]
        # spikes = (v >= 1), matching the reference heaviside exactly
        sA = (np.asarray(r["vA"]) >= 1.0).astype(f32)   # [blk, ch, tl*100+n]
        sB = (np.asarray(r["vB"]) >= 1.0).astype(f32)   # [k, ch, c*100+n]
        sA = sA.reshape(W // 8, 128, 8, F, BL)
        sA = sA.transpose(1, 0, 2, 3, 4).reshape(128, W, F, BL)
        sB = sB.reshape(CH, 128, PB, F, BL)
        sB = sB.transpose(1, 2, 0, 3, 4).reshape(128, W, F, BL)
        out = (sA[:C] + sA[C:] + sB[:C] + sB[C:])   # [C, t, f, b]
        outs.append(out.transpose(3, 0, 2, 1))      # [b, C, f, t]
    return np.ascontiguousarray(np.concatenate(outs, axis=0), dtype=f32)


def kernel(inputs, w3, b3, w7, b7):
    nc = _get_nc()
    per_core = _host_prep(
        np.asarray(inputs, dtype=np.float32),
        np.asarray(w3, dtype=np.float32),
        np.asarray(b3, dtype=np.float32),
        np.asarray(w7, dtype=np.float32),
        np.asarray(b7, dtype=np.float32),
    )
    res = bass_utils.run_bass_kernel_spmd(
        nc, per_core, core_ids=list(range(NCORES))
    )
    LAST_RESULT[0] = res
    return _host_post(res)


# revision 14
# speedup vs baseline: 1.0709x; 1.0194x over previous
"""Trainium2 Bass kernel for nn_DynamicReceptiveEncoder (v2).

Structure (per core, 4 of 32 batch elements):
  PE  : conv3+conv7 as one K=50 matmul (bias via ones row, f32) feeding the
        A-side scan via PSUM; diff-conv (K=49, fp32r, x-diff done on host,
        1.25 threshold-normalization folded into weights) feeding B-side.
  GPS : exact 512-step LIF scan for the two raw-conv neurons (tau 20/50,
        long subthreshold memory - cannot be time-chunked), one
        scalar_tensor_tensor pair per step, reading conv PSUM directly.
  DVE : time-chunked LIF scan for the two |diff| neurons (tau 2/0.91,
        state forgets within ~10 steps): 8 chunks x (64+16) steps
        processed as 800 columns per instruction.
  ACT : |.| eviction of diff-conv PSUM; Sign(v-1) spike masks for both
        sides (bf16) which are DMA'd to DRAM.
  Host: im2col staging matrices (so device DMA is wide contiguous block
        copy), final spike summation across the four neuron masks.
"""

import sys

sys.path.insert(0, "/opt/trn_rl_repo")

import numpy as np

import concourse.bass as bass
import concourse.mybir as mybir
from concourse.tile import TileContext
from concourse import bass_utils

AL = mybir.AluOpType
AF = mybir.ActivationFunctionType
F32 = mybir.dt.float32
F32R = mybir.dt.float32r
BF16 = mybir.dt.bfloat16

# ---------------------------------------------------------------------------
# Patches for this walrus build (max ONE sync wait per instruction) and for
# the missing NTFF profile hook module.
# ---------------------------------------------------------------------------
import concourse.tile as _tile
from concourse.vector_clock import ScopedClock as _ScopedClock

_wsplit_counter = [0]


def _patched_drain_and_barrier(self, tick_clock, wait_clock):
    nc = self.nc
    drain_inst = nc.sync.drain()
    wait_clock.add_sem_waits(
        drain_inst.ins, _ScopedClock({None: tick_clock.global_clock})
    )
    si = drain_inst.ins.sync_info
    waits = list(si.on_wait) if si is not None else []
    if len(waits) > 1:
        updates = list(si.on_update) if si is not None else []
        drain_inst.ins.sync_info = mybir.SyncInfo(on_wait=[], on_update=updates)
        for w in waits:
            nop_inst = nc.sync.nop(nofuse=True)
            nop_inst.ins.sync_info = mybir.SyncInfo(on_wait=[w], on_update=[])

    nc.all_engine_barrier()
    assert self.sems is not None
    popped = nc._tile_sem_poison_stack.pop()
    assert popped is self._sem_poison
    nc.clear_and_free_semaphores(list(self.sems.allocated().values()))
    nc.all_engine_barrier()


_tile.TileContext._drain_and_barrier = _patched_drain_and_barrier


def _split_multi_waits(nc, max_waits=1):
    for f in nc.m.functions:
        for bb in f.blocks:
            insts = bb.instructions
            i = 0
            while i < len(insts):
                inst = insts[i]
                si = inst.sync_info
                if si is not None and len(si.on_wait) > max_waits:
                    waits = list(si.on_wait)
                    extra, keep = waits[:-max_waits], waits[-max_waits:]
                    inst.sync_info = mybir.SyncInfo(
                        on_wait=keep, on_update=list(si.on_update)
                    )
                    for w in extra:
                        _wsplit_counter[0] += 1
                        nop = mybir.InstNoOp(
                            name=f"wsplit_{_wsplit_counter[0]}", ins=[], outs=[]
                        )
                        nop.engine = inst.engine
                        nop.sync_info = mybir.SyncInfo(on_wait=[w], on_update=[])
                        insts.insert(i, nop)
                        i += 1
                i += 1


def _install_ntff_hook():
    import contextlib, ctypes, types

    try:
        lib = ctypes.CDLL("/opt/axon/libaxon_pjrt.so")
    except OSError:
        return
    if not hasattr(lib, "axon_start_nrt_profile"):
        return
    lib.axon_start_nrt_profile.argtypes = [
        ctypes.POINTER(ctypes.c_int64),
        ctypes.c_size_t,
    ]
    lib.axon_start_nrt_profile.restype = ctypes.c_int64
    lib.axon_stop_nrt_profile.argtypes = [ctypes.c_char_p]
    lib.axon_stop_nrt_profile.restype = ctypes.c_int64

    @contextlib.contextmanager
    def _hook(output_dir, device_ids):
        import jax

        jax.devices()
        if device_ids:
            ids = (ctypes.c_int64 * len(device_ids))(*device_ids)
            rc = lib.axon_start_nrt_profile(ids, len(device_ids))
        else:
            rc = lib.axon_start_nrt_profile(None, 0)
        if rc != 0:
            raise RuntimeError(f"axon_start_nrt_profile rc={rc}")
        try:
            yield
        finally:
            lib.axon_stop_nrt_profile(str(output_dir).encode())

    mod = types.ModuleType("antenv.axon_hooks")
    holder = [_hook]
    mod.set_axon_ntff_profile_hook = lambda h: holder.__setitem__(0, h)
    mod.get_axon_ntff_profile_hook = lambda: holder[0]
    sys.modules["antenv.axon_hooks"] = mod
    try:
        import antenv

        antenv.axon_hooks = mod
    except ImportError:
        pass


_install_ntff_hook()

# ---------------------------------------------------------------------------
# Problem constants
# ---------------------------------------------------------------------------
B, W, F, C = 32, 512, 25, 64
NCORES = 8
BL = B // NCORES            # 4 batch elements per core
NBF = BL * F                # 100 (f, b) columns
KA = 50                     # 49 taps + bias/ones row
KD = 49                     # diff-conv taps only (bias cancels)

PB = 8                      # B-side time chunks
CH = W // PB                # 64 steps per chunk
LB = 16                     # B-side warmup steps
SB = CH + LB                # 80 sequential B steps
NB = PB * NBF               # 800 B-side columns per step

TWA = 16                    # A-side staging window (steps)
TWD = 8                     # B-side staging window (B-steps, SB=88 -> 11 windows)

TAU = (20.0, 50.0, 2.0, 0.91)
ALPHA = tuple(np.float32(1.0 - 1.0 / t) for t in TAU)


def _build_nc():
    nc = bass.Bass()
    patA = nc.dram_tensor("patA", [KA, W * NBF], F32, kind="ExternalInput")
    patD = nc.dram_tensor("patD", [KD, SB * NB], F32R, kind="ExternalInput")
    wAt = nc.dram_tensor("wA", [KA, 128], F32, kind="ExternalInput")
    wDt = nc.dram_tensor("wD", [KD, 128], F32R, kind="ExternalInput")
    alAt = nc.dram_tensor("alphaA", [128, 1], F32, kind="ExternalInput")
    alBt = nc.dram_tensor("alphaB", [128, 1], F32, kind="ExternalInput")
    vAo = nc.dram_tensor("vA", [W // 8, 128, 8 * NBF], F32, kind="ExternalOutput")
    vBo = nc.dram_tensor("vB", [CH, 128, NB], F32, kind="ExternalOutput")

    with TileContext(nc) as tc:
        with (
            tc.tile_pool(name="consts", bufs=1) as cpool,
            tc.tile_pool(name="vA", bufs=3) as vApool,
            tc.tile_pool(name="vB", bufs=3) as vBpool,
            tc.tile_pool(name="xA", bufs=3) as xApool,
            tc.tile_pool(name="xB", bufs=3) as xBpool,
            tc.tile_pool(name="psA", bufs=2, space="PSUM") as psApool,
            tc.tile_pool(name="psB", bufs=2, space="PSUM") as psBpool,
        ):
            wA_sb = cpool.tile([KA, 128], F32, name="wA_sb")
            wD_sb = cpool.tile([KD, 128], F32R, name="wD_sb")
            alA = cpool.tile([128, 1], F32, name="alA")
            alB = cpool.tile([128, 1], F32, name="alB")
            wAst = cpool.tile([128, NBF], F32, name="wAst")
            nc.vector.memset(wAst[:], 0.0)
            wBst = cpool.tile([128, NB], F32, name="wBst")
            nc.vector.memset(wBst[:], 0.0)

            # staging buffers (persistent, manual rotation; Tile serializes
            # WAR on reuse)
            rhsA = [cpool.tile([KA, TWA * NBF], F32, name=f"rhsA{i}")
                    for i in range(3)]
            rhsD = [cpool.tile([KD, TWD * NB], F32R, name=f"rhsD{i}")
                    for i in range(3)]

            def stage_A(w):
                eng = nc.sync if w % 2 == 0 else nc.scalar
                eng.dma_start(
                    rhsA[w % 3][:],
                    patA[:, w * TWA * NBF:(w + 1) * TWA * NBF],
                )

            def stage_D(w):
                nc.gpsimd.dma_start(
                    rhsD[w % 3][:],
                    patD[:, w * TWD * NB:(w + 1) * TWD * NB],
                )

            psA_tiles = {}
            xA_tiles = {}

            def conv_A(g):
                # conv for steps 4g..4g+3 (N=400), full fp32 for exactness;
                # two groups share one 2-bank PSUM tile (offsets 0 and 512)
                p = g // 2
                if g % 2 == 0:
                    psA_tiles[p] = psApool.tile([128, 1024], F32, name="psA")
                ps = psA_tiles[p]
                w = g // 4
                sl = (g % 4) * 4 * NBF
                nc.tensor.matmul(
                    ps[:, (g % 2) * 512:(g % 2) * 512 + 4 * NBF],
                    wA_sb[:],
                    rhsA[w % 3][:, sl:sl + 4 * NBF],
                    start=True,
                    stop=True,
                )

            def evict_A(p):
                # PSUM -> SBUF for steps 8p..8p+7, one ACT instruction
                ps = psA_tiles.pop(p)
                xA = xApool.tile([128, 8 * NBF], F32, name="xA")
                xA_tiles[p] = xA
                src_ap = bass.AP(
                    tensor=ps.tensor,
                    offset=ps.offset,
                    ap=[list(ps.ap[0]), [512, 2], [1, 4 * NBF]],
                )
                dst_ap = bass.AP(
                    tensor=xA.tensor,
                    offset=xA.offset,
                    ap=[list(xA.ap[0]), [4 * NBF, 2], [1, 4 * NBF]],
                )
                nc.scalar.activation(dst_ap, src_ap, AF.Identity,
                                     bias=0.0, scale=1.0)

            psB_tiles = {}

            def conv_B(k):
                # diff-conv for B-step k (N=800 as 2x400 in one 2-bank tile)
                w = k // TWD
                sl = (k % TWD) * NB
                ps = psBpool.tile([128, 1024], F32, name="psB")
                psB_tiles[k] = ps
                for h in range(2):
                    nc.tensor.matmul(
                        ps[:, h * 512:h * 512 + 400],
                        wD_sb[:],
                        rhsD[w % 3][:, sl + h * 400: sl + (h + 1) * 400],
                        start=True,
                        stop=True,
                    )

            vA_tiles = {}
            vB_tiles = {}

            def scan_A(t):
                blk = t // 8
                if t % 8 == 0:
                    vA_tiles[blk] = vApool.tile([128, 8 * NBF], F32, name="vA")
                vA = vA_tiles[blk]
                sl = (t % 8) * NBF
                xA = xA_tiles[t // 8]
                nc.vector.scalar_tensor_tensor(
                    out=vA[:, sl:sl + NBF],
                    in0=wAst[:],
                    scalar=alA[:],
                    in1=xA[:, sl:sl + NBF],
                    op0=AL.mult,
                    op1=AL.add,
                )
                nc.vector.scalar_tensor_tensor(
                    out=wAst[:],
                    in0=vA[:, sl:sl + NBF],
                    scalar=1.0,
                    in1=vA[:, sl:sl + NBF],
                    op0=AL.is_lt,
                    op1=AL.mult,
                )
                if t % 8 == 7:
                    xA_tiles.pop(blk, None)
                    eng = nc.sync if blk % 2 == 0 else nc.scalar
                    eng.dma_start(vAo[blk], vA[:])

            pendingB = []

            def scan_B(k):
                # evict runs now (ACT); the 4 DVE scan pieces are queued and
                # dripped one-per-A-step to avoid head-of-line blocking of
                # the serial A chain on the in-order DVE queue.
                ps = psB_tiles.pop(k)
                xB = xBpool.tile([128, NB], F32, name="xB")
                src_ap = bass.AP(
                    tensor=ps.tensor,
                    offset=ps.offset,
                    ap=[list(ps.ap[0]), [512, 2], [1, 400]],
                )
                dst_ap = bass.AP(
                    tensor=xB.tensor,
                    offset=xB.offset,
                    ap=[list(xB.ap[0]), [400, 2], [1, 400]],
                )
                nc.scalar.activation(dst_ap, src_ap, AF.Abs,
                                     bias=0.0, scale=1.0)
                vB = vBpool.tile([128, NB], F32, name="vB")
                vB_tiles[k] = vB
                H = NB // 2

                def vpiece(h):
                    def go():
                        if h == 0 and k == LB:
                            nc.vector.memset(xB[:, 0:NBF], 0.0)
                        nc.vector.scalar_tensor_tensor(
                            out=vB[:, h * H:(h + 1) * H],
                            in0=wBst[:, h * H:(h + 1) * H],
                            scalar=alB[:],
                            in1=xB[:, h * H:(h + 1) * H],
                            op0=AL.mult,
                            op1=AL.add,
                        )
                    return go

                def wpiece(h):
                    def go():
                        nc.vector.scalar_tensor_tensor(
                            out=wBst[:, h * H:(h + 1) * H],
                            in0=vB[:, h * H:(h + 1) * H],
                            scalar=1.0,
                            in1=vB[:, h * H:(h + 1) * H],
                            op0=AL.is_lt,
                            op1=AL.mult,
                        )
                        if h == 1 and k >= LB:
                            eng = nc.scalar if k % 2 == 0 else nc.sync
                            eng.dma_start(vBo[k - LB], vB[:])
                    return go

                pendingB.extend([vpiece(0), vpiece(1), wpiece(0), wpiece(1)])

            # prologue: critical-path DMAs first (first A window + weights
            # + alphas), fanned across the three DMA queues, then the rest
            stage_A(0)                            # sync queue
            stage_D(0)                            # gpsimd queue
            stage_A(1)                            # scalar queue
            nc.sync.dma_start(wA_sb[:], wAt[:])
            nc.scalar.dma_start(alA[:], alAt[:])
            nc.scalar.dma_start(alB[:], alBt[:])
            nc.sync.dma_start(wD_sb[:], wDt[:])
            stage_D(1)
            for g in range(4):
                conv_A(g)
            evict_A(0)
            conv_B(0)

            kB = 0
            for t in range(W):
                if t % TWA == 0 and t // TWA + 2 < W // TWA:
                    stage_A(t // TWA + 2)
                if t % 8 == 0:
                    p = t // 8
                    for g in (2 * p + 4, 2 * p + 5):
                        if g < 128:
                            conv_A(g)
                    if p + 1 < 64:
                        evict_A(p + 1)
                scan_A(t)
                k_target = ((t + 1) * SB) // W
                while kB < min(k_target, SB):
                    k = kB
                    if k % TWD == 0 and k // TWD + 2 < SB // TWD:
                        stage_D(k // TWD + 2)
                    if k + 1 < SB:
                        conv_B(k + 1)
                    scan_B(k)
                    kB += 1
                if pendingB:
                    pendingB.pop(0)()
            while pendingB:
                pendingB.pop(0)()

    _split_multi_waits(nc)
    return nc


_NC_CACHE = [None]
LAST_RESULT = [None]


def _get_nc():
    if _NC_CACHE[0] is None:
        _NC_CACHE[0] = _build_nc()
    return _NC_CACHE[0]


def _host_prep(inputs, w3, b3, w7, b7):
    f32 = np.float32
    swv = np.lib.stride_tricks.sliding_window_view

    # weight matrices [K, 128]; cols 0:64 conv3 channels, 64:128 conv7
    wA = np.zeros((KA, 128), dtype=f32)
    w3r = w3.reshape(C, 3, 3)
    w7r = w7.reshape(C, 7, 7)
    for i in range(3):
        for j in range(3):
            wA[(i + 2) * 7 + (j + 2), 0:C] = w3r[:, i, j]
    for i in range(7):
        for j in range(7):
            wA[i * 7 + j, C:128] = w7r[:, i, j]
    wA[49, 0:C] = b3
    wA[49, C:128] = b7
    wD = (wA[:49] * f32(1.25)).astype(f32)   # threshold-normalized diff conv

    alphaA = np.full((128, 1), ALPHA[0], dtype=f32)
    alphaA[C:, 0] = ALPHA[1]
    alphaB = np.full((128, 1), ALPHA[2], dtype=f32)
    alphaB[C:, 0] = ALPHA[3]

    per_core = []
    for ci in range(NCORES):
        xc = inputs[ci * BL:(ci + 1) * BL]          # [4, 512, 25] (b, t, f)

        # A-side im2col: patA[(i*7+j), t*100 + f*4 + b] = xpad[b, t+j, f+i]
        xp = np.zeros((BL, W + 6, F + 6), dtype=f32)
        xp[:, 3:3 + W, 3:3 + F] = xc
        v = swv(xp, (7, 7), axis=(1, 2))            # [b, t, f, j, i]
        pA = np.ascontiguousarray(v.transpose(4, 3, 1, 2, 0))  # [i,j,t,f,b]
        patA = np.empty((KA, W * NBF), dtype=f32)
        patA[:49] = pA.reshape(49, W * NBF)
        patA[49] = 1.0                               # bias row

        # B-side: temporal diff on the padded domain.
        # xd[tau] = x[tau] - x[tau-1] for tau in [0, 512], xd[512] = -x[511]
        xdp = np.zeros((BL, LB + 3 + W + 3, F + 6), dtype=f32)
        xd = np.diff(xc, axis=1, prepend=np.zeros_like(xc[:, :1]))
        xdp[:, LB + 3:LB + 3 + W, 3:3 + F] = xd
        xdp[:, LB + 3 + W, 3:3 + F] = -xc[:, -1]
        vD = swv(xdp, (7, 7), axis=(1, 2))          # [b, w0, f, j, i]
        pD = np.ascontiguousarray(vD.transpose(4, 3, 1, 2, 0))  # [i,j,w0,f,b]
        pD = pD.reshape(49, LB + W - CH + CH, F, BL) if False else pD
        # columns: (k, c, f, b) with w0 = c*64 + k
        idx = (np.arange(PB)[None, :] * CH + np.arange(SB)[:, None])  # [k, c]
        patD = np.ascontiguousarray(
            pD.reshape(49, -1, F, BL)[:, idx.reshape(-1)]
        ).reshape(KD, SB * NB)

        per_core.append({
            "patA": patA,
            "patD": patD,
            "wA": wA,
            "wD": wD,
            "alphaA": alphaA,
            "alphaB": alphaB,
        })
    return per_core


def _host_post(res):
    f32 = np.float32
    outs = []
    for ci in range(NCORES):
        r = res.results[ci]
        # spikes = (v >= 1), matching the reference heaviside exactly
        sA = (np.asarray(r["vA"]) >= 1.0).astype(f32)   # [blk, ch, tl*100+n]
        sB = (np.asarray(r["vB"]) >= 1.0).astype(f32)   # [k, ch, c*100+n]
        sA = sA.reshape(W // 8, 128, 8, F, BL)
        sA = sA.transpose(1, 0, 2, 3, 4).reshape(128, W, F, BL)
        sB = sB.reshape(CH, 128, PB, F, BL)
        sB = sB.transpose(1, 2, 0, 3, 4).reshape(128, W, F, BL)
        out = (sA[:C] + sA[C:] + sB[:C] + sB[C:])   # [C, t, f, b]
        outs.append(out.transpose(3, 0, 2, 1))      # [b, C, f, t]
    return np.ascontiguousarray(np.concatenate(outs, axis=0), dtype=f32)


def kernel(inputs, w3, b3, w7, b7):
    nc = _get_nc()
    per_core = _host_prep(
        np.asarray(inputs, dtype=np.float32),
        np.asarray(w3, dtype=np.float32),
        np.asarray(b3, dtype=np.float32),
        np.asarray(w7, dtype=np.float32),
        np.asarray(b7, dtype=np.float32),
    )
    res = bass_utils.run_bass_kernel_spmd(
        nc, per_core, core_ids=list(range(NCORES))
    )
    LAST_RESULT[0] = res
    return _host_post(res)
